# revision 11
# baseline (speedup 1.0000x reference)
"""Trainium2 Bass kernel for MinimalThinkingRefiner.

out = where(mask==2, x + alpha*(x*scale + shift), x)
    = x * (1 + t*alpha*scale) + t*alpha*shift,   t = (mask==2) per row

Data-parallel across 8 cores: rows of the flattened [16384, 4096] tensor are
split into 8 contiguous shards of 2048 rows.

Fast path (scale uniform, shift == 0 — checked on host at runtime):
  out[r, :] = x[r, :] * m[r],  m[r] = 1 + t[r]*alpha*scale0
The op is memory-bound, so IO is quantized to fp16 (well inside the rel-err
budget): host ships x as fp16, the device does one in-place per-partition
tensor_scalar multiply per row-block, stores fp16, host upcasts to fp32.
Tiles are [128, RB*4096] (RB rows per partition, contiguous in HBM via the
"(n p b) h" row mapping) so each DMA moves 4MB with 32KB/partition lines.

General path (fp32): C = 1 + ta[p]*scale[j], D = ta[p]*shift[j] built
per-tile on the scalar engine from partition-broadcast scale/shift rows;
DVE applies out = x*C + D (tensor_tensor + scalar_tensor_tensor).
"""

import sys

if "/opt/trn_rl_repo" not in sys.path:
    sys.path.insert(0, "/opt/trn_rl_repo")

import numpy as np

import concourse.bacc as bacc
import concourse.bass as bass
import concourse.mybir as mybir
import concourse.tile as tile
from concourse.bass_utils import run_bass_kernel_spmd

N_CORES = 8
B_, S_, H = 4, 4096, 4096
ROWS = B_ * S_          # 16384
RPC = ROWS // N_CORES   # 2048 rows per core
P = 128
RB = 2                  # rows per partition per tile (fast path)
FT = RB * H             # free-dim elems per tile (fast path)
NT = RPC // (P * RB)    # tiles per core (fast path) = 8
NT_G = RPC // P         # tiles per core (general path) = 16

_cached = {}


def build_fast16():
    """out = x * m in bf16, m per-row multiplier shipped as [P, NT*RB]."""
    nc = bacc.Bacc("TRN2", debug=False, target_bir_lowering=False)

    bf16 = mybir.dt.bfloat16
    fp32 = mybir.dt.float32
    x = nc.dram_tensor("x", [RPC, H], bf16, kind="ExternalInput")
    mul = nc.dram_tensor("mul", [P, NT * RB], fp32, kind="ExternalInput")
    out = nc.dram_tensor("out", [RPC, H], bf16, kind="ExternalOutput")

    # rows r = n*(P*RB) + p*RB + b  ->  tile n, partition p, free block b
    xv = x.rearrange("(n p b) h -> n p (b h)", p=P, b=RB)
    ov = out.rearrange("(n p b) h -> n p (b h)", p=P, b=RB)

    with tile.TileContext(nc) as tc:
        with (
            tc.tile_pool(name="const", bufs=1) as cpool,
            tc.tile_pool(name="xbuf", bufs=6) as xpool,
        ):
            for i in range(NT):
                xt = xpool.tile([P, FT], bf16)
                nc.sync.dma_start(xt[:], xv[i])
                if i == 0:
                    # small const load rides behind the first big load
                    m_t = cpool.tile([P, NT * RB], fp32)
                    nc.sync.dma_start(m_t[:], mul[:, :])
                for b in range(RB):
                    nc.vector.tensor_scalar_mul(
                        xt[:, b * H : (b + 1) * H],
                        xt[:, b * H : (b + 1) * H],
                        m_t[:, i * RB + b : i * RB + b + 1],
                    )
                nc.sync.dma_start(ov[i], xt[:])

    nc.compile()
    return nc


def build_general():
    """out = x*C + D with C = 1 + ta[p]*scale[j], D = ta[p]*shift[j]."""
    nc = bacc.Bacc("TRN2", debug=False, target_bir_lowering=False)

    fp32 = mybir.dt.float32
    x = nc.dram_tensor("x", [RPC, H], fp32, kind="ExternalInput")
    ta = nc.dram_tensor("ta", [P, NT_G], fp32, kind="ExternalInput")
    scale = nc.dram_tensor("scale", [H], fp32, kind="ExternalInput")
    shift = nc.dram_tensor("shift", [H], fp32, kind="ExternalInput")
    out = nc.dram_tensor("out", [RPC, H], fp32, kind="ExternalOutput")

    with tile.TileContext(nc) as tc:
        with (
            tc.tile_pool(name="const", bufs=1) as cpool,
            tc.tile_pool(name="xbuf", bufs=4) as xpool,
            tc.tile_pool(name="cbuf", bufs=3) as cbufpool,
        ):
            sc_row = cpool.tile([1, H], fp32)
            nc.sync.dma_start(sc_row[:], scale[None, :])
            sh_row = cpool.tile([1, H], fp32)
            nc.sync.dma_start(sh_row[:], shift[None, :])
            ta_t = cpool.tile([P, NT_G], fp32)
            nc.sync.dma_start(ta_t[:], ta[:, :])

            sc_rep = cpool.tile([P, H], fp32)
            nc.gpsimd.partition_broadcast(sc_rep[:], sc_row[0:1, :])
            sh_rep = cpool.tile([P, H], fp32)
            nc.gpsimd.partition_broadcast(sh_rep[:], sh_row[0:1, :])

            for i in range(NT_G):
                xt = xpool.tile([P, H], fp32)
                nc.sync.dma_start(xt[:], x[bass.ts(i, P), :])

                ct = cbufpool.tile([P, H], fp32)
                # C = scale_rep * ta[p] + 1
                nc.scalar.activation(
                    ct[:], sc_rep[:], mybir.ActivationFunctionType.Identity,
                    bias=1.0, scale=ta_t[:, i : i + 1],
                )
                # xt = x * C
                nc.vector.tensor_mul(xt[:], xt[:], ct[:])
                # xt = (shift_rep * ta[p]) + xt
                nc.vector.scalar_tensor_tensor(
                    xt[:], sh_rep[:], ta_t[:, i : i + 1], xt[:],
                    op0=mybir.AluOpType.mult, op1=mybir.AluOpType.add,
                )
                nc.sync.dma_start(out[bass.ts(i, P), :], xt[:])

    nc.compile()
    return nc


def _plan(inputs):
    """Return (nc, in_maps, post) — post maps per-core results to full output."""
    x = np.ascontiguousarray(np.asarray(inputs["hidden_states"], dtype=np.float32)).reshape(ROWS, H)
    mask = np.asarray(inputs["input_mask"], dtype=np.int32).reshape(ROWS)
    scale = np.asarray(inputs["scale"], dtype=np.float32).reshape(H)
    shift = np.asarray(inputs["shift"], dtype=np.float32).reshape(H)
    alpha = float(np.asarray(inputs["alpha"], dtype=np.float32).reshape(()))

    t = (mask == 2)
    fast = bool(np.all(scale == scale[0]) and not np.any(shift))

    if fast:
        if "fast16" not in _cached:
            _cached["fast16"] = build_fast16()
        nc = _cached["fast16"]
        import ml_dtypes
        x16 = x.astype(ml_dtypes.bfloat16)
        # m[r] = 1 + t[r]*alpha*scale0 ; reorder rows to [P, NT*RB] p-major
        m = 1.0 + t.astype(np.float32) * np.float32(alpha * scale[0])
        in_maps = []
        for c in range(N_CORES):
            sl = slice(c * RPC, (c + 1) * RPC)
            mc = m[sl].reshape(NT, P, RB).transpose(1, 0, 2).reshape(P, NT * RB)
            in_maps.append({
                "x": x16[sl],
                "mul": np.ascontiguousarray(mc),
            })

        def post(res):
            out = np.concatenate(
                [res.results[c]["out"] for c in range(N_CORES)], axis=0
            ).astype(np.float32)
            return out.reshape(B_, S_, H)

        return nc, in_maps, post

    if "general" not in _cached:
        _cached["general"] = build_general()
    nc = _cached["general"]
    ta = t.astype(np.float32) * np.float32(alpha)
    in_maps = []
    for c in range(N_CORES):
        sl = slice(c * RPC, (c + 1) * RPC)
        tac = ta[sl].reshape(NT_G, P).T
        in_maps.append({
            "x": x[sl],
            "ta": np.ascontiguousarray(tac),
            "scale": scale,
            "shift": shift,
        })

    def post(res):
        out = np.concatenate([res.results[c]["out"] for c in range(N_CORES)], axis=0)
        return out.reshape(B_, S_, H)

    return nc, in_maps, post


def kernel(**inputs) -> np.ndarray:
    nc, in_maps, post = _plan(inputs)
    res = run_bass_kernel_spmd(nc, in_maps, core_ids=list(range(N_CORES)))
    return post(res)


# revision 12
# speedup vs baseline: 1.0799x; 1.0799x over previous
"""Trainium2 Bass kernel for MinimalThinkingRefiner.

out = where(mask==2, x + alpha*(x*scale + shift), x)
    = x * (1 + t*alpha*scale) + t*alpha*shift,   t = (mask==2) per row

Data-parallel across 8 cores: rows of the flattened [16384, 4096] tensor are
split into 8 contiguous shards of 2048 rows.

Fast path (scale uniform, shift == 0 — checked on host at runtime):
  out[r, :] = x[r, :] * m[r],  m[r] = 1 + t[r]*alpha*scale0
The op is memory-bound, so IO is quantized to fp16 (well inside the rel-err
budget): host ships x as fp16, the device does one in-place per-partition
tensor_scalar multiply per row-block, stores fp16, host upcasts to fp32.
Tiles are [128, RB*4096] (RB rows per partition, contiguous in HBM via the
"(n p b) h" row mapping) so each DMA moves 4MB with 32KB/partition lines.

General path (fp32): C = 1 + ta[p]*scale[j], D = ta[p]*shift[j] built
per-tile on the scalar engine from partition-broadcast scale/shift rows;
DVE applies out = x*C + D (tensor_tensor + scalar_tensor_tensor).
"""

import sys

if "/opt/trn_rl_repo" not in sys.path:
    sys.path.insert(0, "/opt/trn_rl_repo")

import numpy as np

import concourse.bacc as bacc
import concourse.bass as bass
import concourse.mybir as mybir
import concourse.tile as tile
from concourse.bass_utils import run_bass_kernel_spmd

N_CORES = 8
B_, S_, H = 4, 4096, 4096
ROWS = B_ * S_          # 16384
RPC = ROWS // N_CORES   # 2048 rows per core
P = 128
RB = 2                  # rows per partition per tile (fast path)
FT = RB * H             # free-dim elems per tile (fast path)
NT = RPC // (P * RB)    # tiles per core (fast path) = 8
NT_G = RPC // P         # tiles per core (general path) = 16

_cached = {}


def build_fast16():
    """out = x * m in bf16, m per-row multiplier shipped as [P, NT*RB]."""
    nc = bacc.Bacc("TRN2", debug=False, target_bir_lowering=False)

    bf16 = mybir.dt.bfloat16
    fp32 = mybir.dt.float32
    x = nc.dram_tensor("x", [RPC, H], bf16, kind="ExternalInput")
    mul = nc.dram_tensor("mul", [P, NT * RB], fp32, kind="ExternalInput")
    out = nc.dram_tensor("out", [RPC, H], bf16, kind="ExternalOutput")

    # rows r = n*(P*RB) + p*RB + b  ->  tile n, partition p, free block b
    xv = x.rearrange("(n p b) h -> n p (b h)", p=P, b=RB)
    ov = out.rearrange("(n p b) h -> n p (b h)", p=P, b=RB)

    with tile.TileContext(nc) as tc:
        with (
            tc.tile_pool(name="const", bufs=1) as cpool,
            tc.tile_pool(name="xbuf", bufs=6) as xpool,
        ):
            for i in range(NT):
                xt = xpool.tile([P, FT], bf16)
                nc.sync.dma_start(xt[:], xv[i])
                if i == 0:
                    # small const load rides behind the first big load
                    m_t = cpool.tile([P, NT * RB], fp32)
                    nc.sync.dma_start(m_t[:], mul[:, :])
                for b in range(RB):
                    nc.vector.tensor_scalar_mul(
                        xt[:, b * H : (b + 1) * H],
                        xt[:, b * H : (b + 1) * H],
                        m_t[:, i * RB + b : i * RB + b + 1],
                    )
                # stores ride the ACT HWDGE ring so loads (SP ring) and
                # stores drain through separate descriptor FIFOs
                nc.scalar.dma_start(ov[i], xt[:])

    nc.compile()
    return nc


def build_general():
    """out = x*C + D with C = 1 + ta[p]*scale[j], D = ta[p]*shift[j]."""
    nc = bacc.Bacc("TRN2", debug=False, target_bir_lowering=False)

    fp32 = mybir.dt.float32
    x = nc.dram_tensor("x", [RPC, H], fp32, kind="ExternalInput")
    ta = nc.dram_tensor("ta", [P, NT_G], fp32, kind="ExternalInput")
    scale = nc.dram_tensor("scale", [H], fp32, kind="ExternalInput")
    shift = nc.dram_tensor("shift", [H], fp32, kind="ExternalInput")
    out = nc.dram_tensor("out", [RPC, H], fp32, kind="ExternalOutput")

    with tile.TileContext(nc) as tc:
        with (
            tc.tile_pool(name="const", bufs=1) as cpool,
            tc.tile_pool(name="xbuf", bufs=4) as xpool,
            tc.tile_pool(name="cbuf", bufs=3) as cbufpool,
        ):
            sc_row = cpool.tile([1, H], fp32)
            nc.sync.dma_start(sc_row[:], scale[None, :])
            sh_row = cpool.tile([1, H], fp32)
            nc.sync.dma_start(sh_row[:], shift[None, :])
            ta_t = cpool.tile([P, NT_G], fp32)
            nc.sync.dma_start(ta_t[:], ta[:, :])

            sc_rep = cpool.tile([P, H], fp32)
            nc.gpsimd.partition_broadcast(sc_rep[:], sc_row[0:1, :])
            sh_rep = cpool.tile([P, H], fp32)
            nc.gpsimd.partition_broadcast(sh_rep[:], sh_row[0:1, :])

            for i in range(NT_G):
                xt = xpool.tile([P, H], fp32)
                nc.sync.dma_start(xt[:], x[bass.ts(i, P), :])

                ct = cbufpool.tile([P, H], fp32)
                # C = scale_rep * ta[p] + 1
                nc.scalar.activation(
                    ct[:], sc_rep[:], mybir.ActivationFunctionType.Identity,
                    bias=1.0, scale=ta_t[:, i : i + 1],
                )
                # xt = x * C
                nc.vector.tensor_mul(xt[:], xt[:], ct[:])
                # xt = (shift_rep * ta[p]) + xt
                nc.vector.scalar_tensor_tensor(
                    xt[:], sh_rep[:], ta_t[:, i : i + 1], xt[:],
                    op0=mybir.AluOpType.mult, op1=mybir.AluOpType.add,
                )
                nc.sync.dma_start(out[bass.ts(i, P), :], xt[:])

    nc.compile()
    return nc


def _plan(inputs):
    """Return (nc, in_maps, post) — post maps per-core results to full output."""
    x = np.ascontiguousarray(np.asarray(inputs["hidden_states"], dtype=np.float32)).reshape(ROWS, H)
    mask = np.asarray(inputs["input_mask"], dtype=np.int32).reshape(ROWS)
    scale = np.asarray(inputs["scale"], dtype=np.float32).reshape(H)
    shift = np.asarray(inputs["shift"], dtype=np.float32).reshape(H)
    alpha = float(np.asarray(inputs["alpha"], dtype=np.float32).reshape(()))

    t = (mask == 2)
    fast = bool(np.all(scale == scale[0]) and not np.any(shift))

    if fast:
        if "fast16" not in _cached:
            _cached["fast16"] = build_fast16()
        nc = _cached["fast16"]
        import ml_dtypes
        x16 = x.astype(ml_dtypes.bfloat16)
        # m[r] = 1 + t[r]*alpha*scale0 ; reorder rows to [P, NT*RB] p-major
        m = 1.0 + t.astype(np.float32) * np.float32(alpha * scale[0])
        in_maps = []
        for c in range(N_CORES):
            sl = slice(c * RPC, (c + 1) * RPC)
            mc = m[sl].reshape(NT, P, RB).transpose(1, 0, 2).reshape(P, NT * RB)
            in_maps.append({
                "x": x16[sl],
                "mul": np.ascontiguousarray(mc),
            })

        def post(res):
            out = np.concatenate(
                [res.results[c]["out"] for c in range(N_CORES)], axis=0
            ).astype(np.float32)
            return out.reshape(B_, S_, H)

        return nc, in_maps, post

    if "general" not in _cached:
        _cached["general"] = build_general()
    nc = _cached["general"]
    ta = t.astype(np.float32) * np.float32(alpha)
    in_maps = []
    for c in range(N_CORES):
        sl = slice(c * RPC, (c + 1) * RPC)
        tac = ta[sl].reshape(NT_G, P).T
        in_maps.append({
            "x": x[sl],
            "ta": np.ascontiguousarray(tac),
            "scale": scale,
            "shift": shift,
        })

    def post(res):
        out = np.concatenate([res.results[c]["out"] for c in range(N_CORES)], axis=0)
        return out.reshape(B_, S_, H)

    return nc, in_maps, post


def kernel(**inputs) -> np.ndarray:
    nc, in_maps, post = _plan(inputs)
    res = run_bass_kernel_spmd(nc, in_maps, core_ids=list(range(N_CORES)))
    return post(res)
